# revision 1
# baseline (speedup 1.0000x reference)
"""Trainium2 Bass kernel for nn_BasicBlock (conv3x3-BN-perelem_act-conv3x3-BN + act shortcut).

Data-parallel over batch: 32 images -> 4 per core x 8 cores.

Per-core layout: each 64x112x112 image is split into top/bottom 56-row halves,
mapped to SBUF partitions 0-63 (top, one per channel) and 64-127 (bottom), so
every elementwise op runs with all 128 lanes and the per-element activation
mask arrays need only a single copy.

Conv3x3 = 9 accumulating K=64 matmuls per 8-row output chunk, run as two
concurrent 64x64 array tiles (tile_position (0,0) for the top half and
(64,64) for the bottom half).

Per-element activation (codes 0..3 = relu/identity/tanh/sigmoid) is computed
as   act(y) = sigmoid(s1*y + s0) * w2 + F
with host-precomputed per-element arrays:
  s1 = {relu: 512, id: 0, tanh: 2, sigmoid: 1}
  s0 = {id: 40, else 0}            (sigmoid(40) == 1 -> identity passes y)
  CD = {tanh: 2, sigmoid: 1, else 0}  (w2 = y, overwritten by CD where CD != 0
                                       via one copy_predicated)
  F  = {tanh: -1, else 0}
BN is folded: scale via the ACT eviction pass (per-partition scale AP),
beta/mean folded into the host-side arrays (zero for this problem's fills).
"""

import os
import sys

sys.path.insert(0, "/opt/trn_rl_repo")

import numpy as np
import ml_dtypes
from contextlib import ExitStack

import concourse.bass as bass
import concourse.bacc as bacc
import concourse.tile as tile
import concourse.mybir as mybir
from concourse.bass_utils import run_bass_kernel_spmd

F16 = np.float16
MDT = mybir.dt.float16
EPS = 1e-5
KREL = 512.0   # sigmoid(KREL*y) ~ step(y) for the relu branch
SAT = 40.0     # sigmoid(40) == 1.0 for the identity branch

B, C, H, W = 32, 64, 112, 112
NCORES = 8
BPC = B // NCORES          # images per core
SEC = H // 2               # rows per half-section (56)
HP, WP = SEC + 2, W + 2    # padded section: 58 x 114
NU = SEC // 8              # 8-row elementwise units per half (7)

TAPS = [(ky, kx) for ky in (-1, 0, 1) for kx in (-1, 0, 1)]

LAST_RESULT = None  # BassKernelResults of the most recent kernel() call


def _split_halves(m):
    """[64, 112, X] -> [128, 56, X]: top rows on partitions 0-63, bottom on 64-127."""
    return np.concatenate([m[:, 0:SEC, :], m[:, SEC:H, :]], axis=0)


def _pad_split_image(img):
    """[64,112,112] fp -> [128, 58, 114] f16 padded split layout (1px halo)."""
    p = np.zeros((C, H + 2, W + 2), np.float32)
    p[:, 1:113, 1:113] = img
    top = p[:, 0:HP, :]
    bot = p[:, SEC:SEC + HP, :]
    return np.concatenate([top, bot], axis=0).astype(F16)


def _mask_arrays(codes, bn_b):
    """codes [C*H*W] int32 -> dict of split-layout [128,56,112] f16 arrays.
    bn_b: per-channel beta-fold (shape [C]) added where needed (F side only
    makes sense for the *final* combine; for the feature layer pass bn_b=0 and
    handle beta via the eviction bias path)."""
    c = codes.reshape(C, H, W)
    s1 = np.select([c == 0, c == 1, c == 2, c == 3], [KREL, 0.0, 2.0, 1.0]).astype(np.float32)
    s0 = np.where(c == 1, SAT, 0.0).astype(np.float32)
    cd = np.select([c == 2, c == 3], [2.0, 1.0], 0.0).astype(np.float32)
    f = np.where(c == 2, -1.0, 0.0).astype(np.float32) + bn_b[:, None, None]
    return {
        "s1": _split_halves(s1).astype(F16),
        "s0": _split_halves(s0).astype(F16),
        "cd": _split_halves(cd).astype(F16),
        "cm": _split_halves((cd != 0).astype(np.float32)).astype(np.uint8),
        "f": _split_halves(f).astype(F16),
    }


def _build_program():
    nc = bacc.Bacc("TRN2", target_bir_lowering=False, debug=False)

    xin = nc.dram_tensor("xin", [BPC, 128, HP, WP], MDT, kind="ExternalInput")
    w1d = nc.dram_tensor("w1", [9, 128, 64], MDT, kind="ExternalInput")
    w2d = nc.dram_tensor("w2", [9, 128, 64], MDT, kind="ExternalInput")
    a1d = nc.dram_tensor("a1", [128, 1], mybir.dt.float32, kind="ExternalInput")
    a2d = nc.dram_tensor("a2", [128, 1], mybir.dt.float32, kind="ExternalInput")
    mnames = ["s1f", "s0f", "cdf", "ff", "s1s", "s0s", "cds", "f2"]
    mdram = {
        k: nc.dram_tensor(k, [128, SEC, W], MDT, kind="ExternalInput") for k in mnames
    }
    for k in ("cmf", "cms"):  # uint8 predicate masks (CopyPredicated needs int dtype)
        mdram[k] = nc.dram_tensor(k, [128, SEC, W], mybir.dt.uint8, kind="ExternalInput")
    outd = nc.dram_tensor("out", [BPC, 128, SEC, W], MDT, kind="ExternalOutput")

    CP = mybir.ActivationFunctionType.Copy
    SG = mybir.ActivationFunctionType.Sigmoid

    with tile.TileContext(nc) as tc, ExitStack() as ctx:
        wp = ctx.enter_context(tc.tile_pool(name="w", bufs=1))
        mp = ctx.enter_context(tc.tile_pool(name="m", bufs=1))
        xp = ctx.enter_context(tc.tile_pool(name="x", bufs=1))
        hp = ctx.enter_context(tc.tile_pool(name="h", bufs=2))
        ep = ctx.enter_context(tc.tile_pool(name="e", bufs=2))
        op_ = ctx.enter_context(tc.tile_pool(name="o", bufs=2))
        pp = ctx.enter_context(tc.tile_pool(name="ps", bufs=4, space="PSUM"))

        w1t = wp.tile([128, 9, 64], MDT, tag="w1")
        w2t = wp.tile([128, 9, 64], MDT, tag="w2")
        for t in range(9):
            nc.sync.dma_start(w1t[:, t, :], w1d[t, :, :])
            nc.sync.dma_start(w2t[:, t, :], w2d[t, :, :])
        a1t = wp.tile([128, 1], mybir.dt.float32, tag="a1")
        a2t = wp.tile([128, 1], mybir.dt.float32, tag="a2")
        nc.sync.dma_start(a1t[:], a1d[:, :])
        nc.sync.dma_start(a2t[:], a2d[:, :])

        mt = {}
        for k in mnames:
            mt[k] = mp.tile([128, SEC, W], MDT, tag=k, name=k)
        for k in ("cmf", "cms"):
            mt[k] = mp.tile([128, SEC, W], mybir.dt.uint8, tag=k, name=k)
        obs = wp.tile([128, 2], MDT, tag="obs", name="obs")
        obu = wp.tile([128, 2], mybir.dt.uint8, tag="obu", name="obu")
        obg = wp.tile([128, 2], MDT, tag="obg", name="obg")
        # interleave DMA chunks and queue-observers by unit so the in-order
        # DVE only stalls on unit-0 chunks before image 0 starts (the rest
        # stream in behind compute)
        for u in range(NU):
            for k in mt:
                nc.sync.dma_start(mt[k][:, 8 * u:8 * u + 8, :],
                                  mdram[k][:, 8 * u:8 * u + 8, :])
            for k in mt:
                dst = obu if k in ("cmf", "cms") else obs
                nc.vector.tensor_add(dst[0:1, 0:1], mt[k][0:1, 8 * u, 0:1],
                                     mt[k][0:1, 8 * u, 0:1])
                if k in ("ff", "f2", "s0f", "s0s"):
                    nc.gpsimd.tensor_add(obg[0:1, 0:1], mt[k][0:1, 8 * u, 0:1],
                                         mt[k][0:1, 8 * u, 0:1])

        def conv_unit(src, wt, ps, r0):
            """9-tap conv into 2-bank psum tile ps[:, 0:8, 0:112] for output
            rows r0..r0+7 of each half; both halves concurrently."""
            for i in (0, 1):
                for t, (ky, kx) in enumerate(TAPS):
                    rs = r0 + 4 * i + 1 + ky
                    rhs_t = src[0:64, rs:rs + 4, kx + 1:kx + 113]
                    rhs_b = src[64:128, rs:rs + 4, kx + 1:kx + 113]
                    nc.tensor.matmul(
                        ps[0:64, 4 * i:4 * i + 4, 0:112], wt[0:64, t, :], rhs_t,
                        start=(t == 0), stop=(t == 8), tile_position=(0, 0),
                        skip_group_check=True,
                    )
                    nc.tensor.matmul(
                        ps[64:128, 4 * i:4 * i + 4, 0:112], wt[64:128, t, :], rhs_b,
                        start=(t == 0), stop=(t == 8), tile_position=(64, 64),
                        skip_group_check=True,
                    )

        for n in range(BPC):
            xt = xp.tile([128, HP, WP], MDT, tag="xt")
            nc.sync.dma_start(xt[:], xin[n, :, :, :])
            ht = hp.tile([128, HP, WP], MDT, tag="ht")
            if n < 2:
                # borders stay zero across reuses; interior is fully rewritten
                nc.gpsimd.memset(ht[:], 0.0)

            # ---- layer 1: conv1 -> BN1 -> per-element act -> ht
            for u in range(NU):
                r0 = 8 * u
                ps = pp.tile([128, 8, 128], mybir.dt.float32, tag="ps")
                conv_unit(xt, w1t, ps, r0)
                psv = ps[:, :, 0:112]
                y = ep.tile([128, 8, 112], MDT, tag="y", bufs=3)
                if u % 4 == 3:
                    nc.scalar.activation(y[:], psv, CP, scale=a1t[:])
                else:
                    nc.vector.tensor_scalar_mul(y[:], psv, a1t[:])
                xs = ep.tile([128, 8, 112], MDT, tag="xs")
                nc.vector.tensor_mul(xs[:], y[:], mt["s1f"][:, r0:r0 + 8, :])
                xs2 = ep.tile([128, 8, 112], MDT, tag="xs2")
                nc.vector.tensor_add(xs2[:], xs[:], mt["s0f"][:, r0:r0 + 8, :])
                sg = ep.tile([128, 8, 112], MDT, tag="sg", bufs=3)
                nc.scalar.activation(sg[:], xs2[:], SG)
                nc.vector.copy_predicated(
                    y[:], mt["cmf"][:, r0:r0 + 8, :], mt["cdf"][:, r0:r0 + 8, :])
                h2u = ep.tile([128, 8, 112], MDT, tag="h2u")
                nc.vector.tensor_mul(h2u[:], sg[:], y[:])
                hv = ht[:, r0 + 1:r0 + 9, 1:113]
                nc.gpsimd.tensor_add(hv, h2u[:], mt["ff"][:, r0:r0 + 8, :])

            # halo exchange between the two halves of ht (row 56 of the image
            # is the bottom half's first output row; row 55 is the top's last)
            nc.gpsimd.dma_start(ht[0:64, HP - 1, 1:113], ht[64:128, 1, 1:113])
            nc.gpsimd.dma_start(ht[64:128, 0, 1:113], ht[0:64, SEC, 1:113])

            # ---- layer 2: conv2 -> BN2 (+ shortcut act(x)) -> out
            for u in range(NU):
                r0 = 8 * u
                ps = pp.tile([128, 8, 128], mybir.dt.float32, tag="ps")
                conv_unit(ht, w2t, ps, r0)
                psv = ps[:, :, 0:112]
                y2 = ep.tile([128, 8, 112], MDT, tag="y2", bufs=3)
                if u % 4 == 3:
                    nc.scalar.activation(y2[:], psv, CP, scale=a2t[:])
                else:
                    nc.vector.tensor_scalar_mul(y2[:], psv, a2t[:])
                xu = ep.tile([128, 8, 112], MDT, tag="xu")
                nc.sync.dma_start(xu[:], xin[n, :, r0 + 1:r0 + 9, 1:113])
                nc.vector.tensor_add(obs[0:1, 1:2], xu[0:1, 0, 0:1], xu[0:1, 0, 0:1])
                xv = xu[:]
                t1 = ep.tile([128, 8, 112], MDT, tag="t1")
                nc.vector.tensor_mul(t1[:], xv, mt["s1s"][:, r0:r0 + 8, :])
                t2 = ep.tile([128, 8, 112], MDT, tag="t2")
                nc.vector.tensor_add(t2[:], t1[:], mt["s0s"][:, r0:r0 + 8, :])
                sg2 = ep.tile([128, 8, 112], MDT, tag="sg2", bufs=3)
                nc.scalar.activation(sg2[:], t2[:], SG)
                nc.vector.copy_predicated(
                    xv, mt["cms"][:, r0:r0 + 8, :], mt["cds"][:, r0:r0 + 8, :])
                z = ep.tile([128, 8, 112], MDT, tag="z")
                nc.vector.tensor_mul(z[:], sg2[:], xv)
                z2 = ep.tile([128, 8, 112], MDT, tag="z2")
                nc.gpsimd.tensor_add(z2[:], z[:], mt["f2"][:, r0:r0 + 8, :])
                o = op_.tile([128, 8, 112], MDT, tag="o", bufs=3)
                nc.gpsimd.tensor_add(o[:], y2[:], z2[:])
                nc.sync.dma_start(outd[n, :, r0:r0 + 8, :], o[:])

    nc.compile()
    return nc


def kernel(x, conv1_w, conv2_w, gamma1, beta1, mean1, var1,
           gamma2, beta2, mean2, var2, act_codes_feat, act_codes_sc):
    x = np.asarray(x, np.float32)
    a1 = (np.asarray(gamma1) / np.sqrt(np.asarray(var1) + EPS)).astype(np.float32)
    b1 = (np.asarray(beta1) - np.asarray(mean1) * a1).astype(np.float32)
    a2 = (np.asarray(gamma2) / np.sqrt(np.asarray(var2) + EPS)).astype(np.float32)
    b2 = (np.asarray(beta2) - np.asarray(mean2) * a2).astype(np.float32)

    mf = _mask_arrays(np.asarray(act_codes_feat), np.zeros(C, np.float32))
    ms = _mask_arrays(np.asarray(act_codes_sc), b2)

    # beta1 != 0 would need a bias in the L1 eviction; fold what we can and
    # fail loudly otherwise (the benchmark fills use beta=0, mean=0).
    assert np.allclose(b1, 0.0), "beta1/mean1 fold not implemented for nonzero values"

    w1h = np.zeros((9, 128, 64), F16)
    w2h = np.zeros((9, 128, 64), F16)
    for t, (ky, kx) in enumerate(TAPS):
        w1h[t, 0:64] = w1h[t, 64:128] = np.asarray(conv1_w)[:, :, ky + 1, kx + 1].T.astype(F16)
        w2h[t, 0:64] = w2h[t, 64:128] = np.asarray(conv2_w)[:, :, ky + 1, kx + 1].T.astype(F16)

    a1h = np.concatenate([a1, a1]).reshape(128, 1).astype(np.float32)
    a2h = np.concatenate([a2, a2]).reshape(128, 1).astype(np.float32)

    nc = _build_program()

    in_maps = []
    for core in range(NCORES):
        xs = np.stack([
            _pad_split_image(x[core * BPC + i]) for i in range(BPC)
        ])
        in_maps.append({
            "xin": xs,
            "w1": w1h, "w2": w2h, "a1": a1h, "a2": a2h,
            "s1f": mf["s1"], "s0f": mf["s0"], "cdf": mf["cd"], "ff": mf["f"],
            "s1s": ms["s1"], "s0s": ms["s0"], "cds": ms["cd"], "f2": ms["f"],
            "cmf": mf["cm"], "cms": ms["cm"],
        })

    res = run_bass_kernel_spmd(nc, in_maps, core_ids=list(range(NCORES)))
    global LAST_RESULT
    LAST_RESULT = res

    out = np.empty((B, C, H, W), np.float32)
    for core in range(NCORES):
        o = res.results[core]["out"]  # [BPC, 128, 56, 112] f16
        for i in range(BPC):
            img = np.concatenate([o[i, 0:64], o[i, 64:128]], axis=1)
            out[core * BPC + i] = img.astype(np.float32)
    return out


if __name__ == "__main__":
    rng = np.random.default_rng(0)
    inputs = {
        "x": rng.standard_normal((B, C, H, W), np.float32),
        "conv1_w": rng.standard_normal((C, C, 3, 3), np.float32) * 0.05,
        "conv2_w": rng.standard_normal((C, C, 3, 3), np.float32) * 0.05,
        "gamma1": np.ones(C, np.float32), "beta1": np.zeros(C, np.float32),
        "mean1": np.zeros(C, np.float32), "var1": np.ones(C, np.float32),
        "gamma2": np.ones(C, np.float32), "beta2": np.zeros(C, np.float32),
        "mean2": np.zeros(C, np.float32), "var2": np.ones(C, np.float32),
        "act_codes_feat": rng.integers(0, 4, C * H * W).astype(np.int32),
        "act_codes_sc": rng.integers(0, 4, C * H * W).astype(np.int32),
    }
    out = kernel(**inputs)
    print("out", out.shape, out.dtype, float(np.abs(out).max()))



# revision 25
# speedup vs baseline: 2.3460x; 2.3460x over previous
"""Trainium2 Bass kernel for nn_BasicBlock (conv3x3-BN-perelem_act-conv3x3-BN + act shortcut).

Data-parallel over batch: 32 images -> 4 per core x 8 cores.

Per-core layout: each 64x112x112 image is split into top/bottom 56-row halves,
mapped to SBUF partitions 0-63 (top, one per channel) and 64-127 (bottom), so
every elementwise op runs with all 128 lanes and the per-element activation
mask arrays need only a single copy.

Conv3x3 = 9 accumulating K=128 matmuls per 8-row output chunk using
BLOCK-DIAGONAL weights diag(W_tap, W_tap) [128x128]: one full-width matmul
computes both image halves at once (the cost model charges out-free-size only,
so this halves PE time vs two 64x64 array-tile matmuls).

Per-element activation (codes 0..3 = relu/identity/tanh/sigmoid) is computed
as   act(z) = (sigmoid(s1*z) + f) * w2
with host-precomputed per-element arrays:
  s1 = {relu: 512, id: 0, tanh: 2, sigmoid: 1}
  f  = {id: +0.5, tanh: -0.5, else 0}
  w2 = z, overwritten where code in {tanh, sigmoid} with CD = {tanh: 2,
       sigmoid: 1} via one copy_predicated
(identity: (0.5+0.5)*z = z; tanh: (sig(2z)-0.5)*2; relu: step(z)*z.)
BN is folded exactly via the scalar-engine eviction z = Copy(psum*a + b)
with per-channel scale a = gamma/sqrt(var+eps) (Identity act func) and bias b = beta - mean*a.

The shortcut act reads x from the SBUF input tile (no DRAM reload); its
copy_predicated overwrites the input tile in place (conv1 is done with it).
"""

import os
import sys

sys.path.insert(0, "/opt/trn_rl_repo")

import numpy as np
from contextlib import ExitStack

import concourse.bass as bass
import concourse.bacc as bacc
import concourse.tile as tile
import concourse.mybir as mybir
from concourse.bass_utils import run_bass_kernel_spmd

F16 = np.float16
MDT = mybir.dt.float16
EPS = 1e-5
KREL = 512.0   # sigmoid(KREL*z) ~ step(z) for the relu branch

B, C, H, W = 32, 64, 112, 112
NCORES = 8
BPC = B // NCORES          # images per core
SEC = H // 2               # rows per half-section (56)
HP, WP = SEC + 2, W + 2    # padded section: 58 x 114
NU = SEC // 8              # 8-row elementwise units per half (7)

TAPS = [(ky, kx) for ky in (-1, 0, 1) for kx in (-1, 0, 1)]

LAST_RESULT = None  # BassKernelResults of the most recent kernel() call


def _split_halves(m):
    """[64, 112, X] -> [128, 56, X]: top rows on partitions 0-63, bottom on 64-127."""
    return np.concatenate([m[:, 0:SEC, :], m[:, SEC:H, :]], axis=0)


def _pad_split_image(img):
    """[64,112,112] fp -> [128, 58, 114] f16 padded split layout (1px halo)."""
    p = np.zeros((C, H + 2, W + 2), np.float32)
    p[:, 1:113, 1:113] = img
    top = p[:, 0:HP, :]
    bot = p[:, SEC:SEC + HP, :]
    return np.concatenate([top, bot], axis=0).astype(F16)


def _mask_arrays(codes):
    """codes [C*H*W] int32 -> dict of split-layout [128,56,112] arrays."""
    c = codes.reshape(C, H, W)
    s1 = np.select([c == 0, c == 1, c == 2, c == 3], [KREL, 0.0, 2.0, 1.0]).astype(np.float32)
    f = np.select([c == 1, c == 2], [0.5, -0.5], 0.0).astype(np.float32)
    cd = np.select([c == 2, c == 3], [2.0, 1.0], 0.0).astype(np.float32)
    return {
        "s1": _split_halves(s1).astype(F16),
        "f": _split_halves(f).astype(F16),
        "cd": _split_halves(cd).astype(F16),
        "cm": _split_halves((cd != 0).astype(np.float32)).astype(np.uint8),
    }


def _build_program():
    nc = bacc.Bacc("TRN2", target_bir_lowering=False, debug=False)

    xin = nc.dram_tensor("xin", [BPC, 128, HP, WP], MDT, kind="ExternalInput")
    w1d = nc.dram_tensor("w1", [128, 9, 128], MDT, kind="ExternalInput")
    w2d = nc.dram_tensor("w2", [128, 9, 128], MDT, kind="ExternalInput")
    scld = nc.dram_tensor("scl", [128, 4], mybir.dt.float32, kind="ExternalInput")
    mnames = ["s1f", "ff", "cdf", "s1s", "fs", "cds"]
    mdram = {
        k: nc.dram_tensor(k, [128, SEC, W], MDT, kind="ExternalInput") for k in mnames
    }
    for k in ("cmf", "cms"):  # uint8 predicate masks
        mdram[k] = nc.dram_tensor(k, [128, SEC, W], mybir.dt.uint8, kind="ExternalInput")
    outd = nc.dram_tensor("out", [BPC, 128, SEC, W], MDT, kind="ExternalOutput")

    IDN = mybir.ActivationFunctionType.Identity
    SG = mybir.ActivationFunctionType.Sigmoid
    BYP = mybir.AluOpType.bypass
    ADD = mybir.AluOpType.add

    with tile.TileContext(nc) as tc, ExitStack() as ctx:
        wp = ctx.enter_context(tc.tile_pool(name="w", bufs=1))
        mp = ctx.enter_context(tc.tile_pool(name="m", bufs=1))
        xp = ctx.enter_context(tc.tile_pool(name="x", bufs=3))
        hp = ctx.enter_context(tc.tile_pool(name="h", bufs=2))
        ep = ctx.enter_context(tc.tile_pool(name="e", bufs=2))
        op_ = ctx.enter_context(tc.tile_pool(name="o", bufs=3))
        pp = ctx.enter_context(tc.tile_pool(name="ps", bufs=4, space="PSUM"))

        w1t = wp.tile([128, 9, 128], MDT, tag="w1")
        w2t = wp.tile([128, 9, 128], MDT, tag="w2")
        sclt = wp.tile([128, 4], mybir.dt.float32, tag="scl")
        a1t, b1t, a2t, b2t = (sclt[:, i:i + 1] for i in range(4))
        mt = {}
        for k in mnames:
            mt[k] = mp.tile([128, SEC, W], MDT, tag=k, name=k)
        for k in ("cmf", "cms"):
            mt[k] = mp.tile([128, SEC, W], mybir.dt.uint8, tag=k, name=k)

        def conv_unit(src, wt, ps, r0):
            """9-tap block-diag conv into psum ps[:, 0:8, 0:112] for output
            rows r0..r0+7 of both halves at once (K=128, M=128)."""
            for i in (0, 1):
                for t, (ky, kx) in enumerate(TAPS):
                    rs = r0 + 4 * i + 1 + ky
                    nc.tensor.matmul(
                        ps[:, 4 * i:4 * i + 4, 0:112], wt[:, t, :],
                        src[:, rs:rs + 4, kx + 1:kx + 113],
                        start=(t == 0), stop=(t == 8),
                        skip_group_check=True,
                    )

        xts = {}
        hts = {}

        def load_x(n):
            # issued from the Pool queue mid-L2-phase: Pool's in-order engine
            # gates the transfer so it cannot race earlier DMAs on the
            # exclusive DMA-engines device
            if n >= BPC:
                return
            xts[n] = xp.tile([128, HP, WP], MDT, tag="xt", name=f"xt{n}")
            nc.gpsimd.dma_start(xts[n][:], xin[n, :, :, :])

        def phase_l1(n):
            """conv1 -> BN1 -> per-element act -> ht; prefetches xt(n+1)."""
            ht = hp.tile([128, HP, WP], MDT, tag="ht")
            hts[n] = ht
            if n < 2:
                # borders stay zero across buffer reuses; interior rows/cols
                # are fully rewritten every image (halo rows every image)
                nc.gpsimd.memset(ht[:, 0, :], 0.0)
                nc.gpsimd.memset(ht[:, HP - 1, :], 0.0)
                nc.gpsimd.memset(ht[:, :, 0], 0.0)
                nc.gpsimd.memset(ht[:, :, WP - 1], 0.0)
            xt = xts[n]
            # 1-unit emission lag for the chain tail (ht mul): keeps each
            # in-order engine queue from head-of-line blocking on the
            # cross-engine z->xs->sigmoid->(+f)->mul dependency ring
            lag = []

            def flush_l1(item):
                lag.append(item)
                if len(lag) < 2:
                    return
                sgp, zp, rp = lag.pop(0)
                sg2 = ep.tile([128, 8, 112], MDT, tag="sg2", bufs=2)
                nc.vector.tensor_add(sg2[:], sgp[:], mt["ff"][:, rp:rp + 8, :])
                nc.vector.tensor_mul(ht[:, rp + 1:rp + 9, 1:113], sg2[:], zp[:])

            for u in range(NU):
                r0 = 8 * u
                ps = pp.tile([128, 8, 128], mybir.dt.float32, tag="ps")
                conv_unit(xt, w1t, ps, r0)
                psv = ps[:, :, 0:112]
                z = ep.tile([128, 8, 112], MDT, tag="z", bufs=3)
                nc.scalar.activation(z[:], psv, IDN, scale=a1t[:], bias=b1t[:])
                xs = ep.tile([128, 8, 112], MDT, tag="xs")
                nc.vector.tensor_mul(xs[:], z[:], mt["s1f"][:, r0:r0 + 8, :])
                sg = ep.tile([128, 8, 112], MDT, tag="sg", bufs=3)
                nc.scalar.activation(sg[:], xs[:], SG)
                nc.vector.copy_predicated(
                    z[:], mt["cmf"][:, r0:r0 + 8, :], mt["cdf"][:, r0:r0 + 8, :])
                flush_l1((sg, z, r0))
            flush_l1((None, None, None))

            # halo exchange between the two halves of ht (row 56 of the image
            # is the bottom half's first output row; row 55 is the top's last)
            nc.sync.dma_start(ht[0:64, HP - 1, 1:113], ht[64:128, 1, 1:113])
            nc.sync.dma_start(ht[64:128, 0, 1:113], ht[0:64, SEC, 1:113])

        def phase_l2(n):
            """conv2 -> BN2 (+ shortcut act(x)) -> out"""
            ht = hts.pop(n)
            xt = xts.pop(n)
            lag = []

            def flush_l2(item):
                lag.append(item)
                if len(lag) < 2:
                    return
                sgsp, z2p, rp = lag.pop(0)
                xvp = xt[:, rp + 1:rp + 9, 1:113]
                sgs2 = ep.tile([128, 8, 112], MDT, tag="sgs2", bufs=2)
                nc.vector.tensor_add(sgs2[:], sgsp[:], mt["fs"][:, rp:rp + 8, :])
                sc = ep.tile([128, 8, 112], MDT, tag="sc")
                nc.vector.tensor_mul(sc[:], sgs2[:], xvp)
                o = op_.tile([128, 8, 112], MDT, tag="o")
                nc.gpsimd.scalar_tensor_tensor(o[:], z2p[:], 0.0, sc[:], BYP, ADD)
                nc.sync.dma_start(outd[n, :, rp:rp + 8, :], o[:])

            last = (n == BPC - 1)
            for u in range(NU):
                r0 = 8 * u
                xv = xt[:, r0 + 1:r0 + 9, 1:113]
                if last and u == NU - 1:
                    # final unit of the whole program: hoist the
                    # conv-independent shortcut chain ahead of the conv so
                    # only z2 + o + store remain after the last matmul
                    xss = ep.tile([128, 8, 112], MDT, tag="xss")
                    nc.vector.tensor_mul(xss[:], xv, mt["s1s"][:, r0:r0 + 8, :])
                    sgs = ep.tile([128, 8, 112], MDT, tag="sgs", bufs=3)
                    nc.scalar.activation(sgs[:], xss[:], SG)
                    sgs2l = ep.tile([128, 8, 112], MDT, tag="sgs2", bufs=2)
                    nc.vector.tensor_add(sgs2l[:], sgs[:], mt["fs"][:, r0:r0 + 8, :])
                    nc.vector.copy_predicated(
                        xv, mt["cms"][:, r0:r0 + 8, :], mt["cds"][:, r0:r0 + 8, :])
                    scl_ = ep.tile([128, 8, 112], MDT, tag="sc", name="sc_last")
                    nc.vector.tensor_mul(scl_[:], sgs2l[:], xv)
                if last and u == NU - 1:
                    # very last unit: each 4-row group gets its own psum ring
                    # tile (a shared tile would WAR-serialize group 1's
                    # matmuls behind group 0's eviction read) and is evicted
                    # and stored as soon as its 9-tap accumulation completes,
                    # so only a 4-row evict+store chain trails the last matmul
                    for i in (0, 1):
                        psh = pp.tile([128, 8, 128], mybir.dt.float32,
                                      tag="ps", name=f"ps_last{i}")
                        for t, (ky, kx) in enumerate(TAPS):
                            rs = r0 + 4 * i + 1 + ky
                            nc.tensor.matmul(
                                psh[:, 0:4, 0:112], w2t[:, t, :],
                                ht[:, rs:rs + 4, kx + 1:kx + 113],
                                start=(t == 0), stop=(t == 8),
                                skip_group_check=True,
                            )
                        z2h = ep.tile([128, 4, 112], MDT, tag="z2h", bufs=2)
                        nc.scalar.activation(z2h[:], psh[:, 0:4, 0:112],
                                             IDN, scale=a2t[:], bias=b2t[:])
                        oh = op_.tile([128, 4, 112], MDT, tag="oh", bufs=2)
                        nc.vector.tensor_add(oh[:], z2h[:],
                                             scl_[:, 4 * i:4 * i + 4, :])
                        nc.sync.dma_start(
                            outd[n, :, r0 + 4 * i:r0 + 4 * i + 4, :], oh[:])
                    flush_l2((None, None, None))
                    break
                ps = pp.tile([128, 8, 128], mybir.dt.float32, tag="ps")
                conv_unit(ht, w2t, ps, r0)
                psv = ps[:, :, 0:112]
                z2 = ep.tile([128, 8, 112], MDT, tag="z2", bufs=3)
                nc.scalar.activation(z2[:], psv, IDN, scale=a2t[:], bias=b2t[:])
                xss = ep.tile([128, 8, 112], MDT, tag="xss")
                nc.vector.tensor_mul(xss[:], xv, mt["s1s"][:, r0:r0 + 8, :])
                sgs = ep.tile([128, 8, 112], MDT, tag="sgs", bufs=3)
                nc.scalar.activation(sgs[:], xss[:], SG)
                nc.vector.copy_predicated(
                    xv, mt["cms"][:, r0:r0 + 8, :], mt["cds"][:, r0:r0 + 8, :])
                flush_l2((sgs, z2, r0))
                if u == 1:
                    load_x(n + 2)
            if not last:
                flush_l2((None, None, None))

        # startup DMA order matters: the DMA-engines device is exclusive, so
        # issue what the first conv needs (w1, xt0 leading rows) before the
        # bulk mask load. xt(0) is row-chunked so conv unit 0 starts early.
        # first rows of xt0 go on the scalar queue (parallel to SP) so the
        # first conv unit's data and w1 transfer concurrently
        xt0 = xp.tile([128, HP, WP], MDT, tag="xt", name="xt0")
        xts[0] = xt0
        nc.scalar.dma_start(xt0[:, 0:10, :], xin[0, :, 0:10, :])
        nc.sync.dma_start(w1t[:], w1d[:, :, :])
        nc.sync.dma_start(sclt[:], scld[:, :])
        # L1 masks (small leading chunk so unit-0 elementwise starts early)
        # interleaved with the rest of xt0; conv2 weights + shortcut masks last
        nc.sync.dma_start(xt0[:, 10:26, :], xin[0, :, 10:26, :])
        for k in ("s1f", "ff", "cdf", "cmf"):
            nc.sync.dma_start(mt[k][:, 0:8, :], mdram[k][:, 0:8, :])
        nc.sync.dma_start(xt0[:, 26:42, :], xin[0, :, 26:42, :])
        for k in ("s1f", "ff", "cdf", "cmf"):
            nc.sync.dma_start(mt[k][:, 8:32, :], mdram[k][:, 8:32, :])
        nc.sync.dma_start(xt0[:, 42:HP, :], xin[0, :, 42:HP, :])
        xt1 = xp.tile([128, HP, WP], MDT, tag="xt", name="xt1")
        xts[1] = xt1
        nc.sync.dma_start(xt1[:], xin[1, :, :, :])
        for k in ("s1f", "ff", "cdf", "cmf"):
            nc.sync.dma_start(mt[k][:, 32:SEC, :], mdram[k][:, 32:SEC, :])
        nc.sync.dma_start(w2t[:], w2d[:, :, :])
        for r0, r1 in [(0, 28), (28, SEC)]:
            for k in ("s1s", "fs", "cds", "cms"):
                nc.sync.dma_start(mt[k][:, r0:r1, :], mdram[k][:, r0:r1, :])

        # PE p-state warmup: ~3us of scratch matmuls while the first input
        # chunks transfer, so real matmuls start at the full 2.4 GHz rate.
        # Results land in the first psum ring buffer and are never read
        # (the later write-after-write reuse is tracked by the tile pool).
        wu = wp.tile([128, 448], MDT, tag="wu")
        nc.gpsimd.memset(wu[:], 0.0)
        wups = pp.tile([128, 8, 128], mybir.dt.float32, tag="ps", name="warm")
        for i in range(8):
            nc.tensor.matmul(wups[:, 0:4, 0:112], wu[:, 0:128], wu[:, 0:448],
                             start=True, stop=True, skip_group_check=True)

        # software pipeline: keep the PE fed with independent conv work at
        # every L1->L2 boundary (L2(n) waits on ht(n)+halo; L1(n+1) does not)
        phase_l1(0)
        for n in range(BPC):
            if n + 1 < BPC:
                phase_l1(n + 1)
            phase_l2(n)

    nc.compile()
    return nc


def kernel(x, conv1_w, conv2_w, gamma1, beta1, mean1, var1,
           gamma2, beta2, mean2, var2, act_codes_feat, act_codes_sc):
    x = np.asarray(x, np.float32)
    a1 = (np.asarray(gamma1) / np.sqrt(np.asarray(var1) + EPS)).astype(np.float32)
    b1 = (np.asarray(beta1) - np.asarray(mean1) * a1).astype(np.float32)
    a2 = (np.asarray(gamma2) / np.sqrt(np.asarray(var2) + EPS)).astype(np.float32)
    b2 = (np.asarray(beta2) - np.asarray(mean2) * a2).astype(np.float32)

    mf = _mask_arrays(np.asarray(act_codes_feat))
    ms = _mask_arrays(np.asarray(act_codes_sc))

    w1h = np.zeros((9, 128, 128), F16)
    w2h = np.zeros((9, 128, 128), F16)
    for t, (ky, kx) in enumerate(TAPS):
        w1h[t, 0:64, 0:64] = w1h[t, 64:128, 64:128] = \
            np.asarray(conv1_w)[:, :, ky + 1, kx + 1].T.astype(F16)
        w2h[t, 0:64, 0:64] = w2h[t, 64:128, 64:128] = \
            np.asarray(conv2_w)[:, :, ky + 1, kx + 1].T.astype(F16)
    w1h = np.ascontiguousarray(w1h.transpose(1, 0, 2))  # [128, 9, 128]
    w2h = np.ascontiguousarray(w2h.transpose(1, 0, 2))

    dup = lambda v: np.concatenate([v, v]).astype(np.float32)
    sclh = np.stack([dup(a1), dup(b1), dup(a2), dup(b2)], axis=1)  # [128, 4]

    nc = _build_program()

    in_maps = []
    for core in range(NCORES):
        xs = np.stack([
            _pad_split_image(x[core * BPC + i]) for i in range(BPC)
        ])
        in_maps.append({
            "xin": xs,
            "w1": w1h, "w2": w2h, "scl": sclh,
            "s1f": mf["s1"], "ff": mf["f"], "cdf": mf["cd"], "cmf": mf["cm"],
            "s1s": ms["s1"], "fs": ms["f"], "cds": ms["cd"], "cms": ms["cm"],
        })

    res = run_bass_kernel_spmd(nc, in_maps, core_ids=list(range(NCORES)))
    global LAST_RESULT
    LAST_RESULT = res

    out = np.empty((B, C, H, W), np.float32)
    for core in range(NCORES):
        o = res.results[core]["out"]  # [BPC, 128, 56, 112] f16
        for i in range(BPC):
            img = np.concatenate([o[i, 0:64], o[i, 64:128]], axis=1)
            out[core * BPC + i] = img.astype(np.float32)
    return out


if __name__ == "__main__":
    rng = np.random.default_rng(0)
    inputs = {
        "x": rng.standard_normal((B, C, H, W)).astype(np.float32),
        "conv1_w": (rng.standard_normal((C, C, 3, 3)) * 0.05).astype(np.float32),
        "conv2_w": (rng.standard_normal((C, C, 3, 3)) * 0.05).astype(np.float32),
        "gamma1": np.ones(C, np.float32), "beta1": np.zeros(C, np.float32),
        "mean1": np.zeros(C, np.float32), "var1": np.ones(C, np.float32),
        "gamma2": np.ones(C, np.float32), "beta2": np.zeros(C, np.float32),
        "mean2": np.zeros(C, np.float32), "var2": np.ones(C, np.float32),
        "act_codes_feat": rng.integers(0, 4, C * H * W).astype(np.int32),
        "act_codes_sc": rng.integers(0, 4, C * H * W).astype(np.int32),
    }
    out = kernel(**inputs)
    print("out", out.shape, out.dtype, float(np.abs(out).max()))
